# revision 11
# baseline (speedup 1.0000x reference)
"""Trainium2 Bass kernel for nn_CLFBlock (linear -> LIF scan -> linear -> T-mean -> log_softmax).

Self-contained: hardcodes shapes T=32, B=512, D=1024, C=1000 and data-parallel
sharding of the batch dim across 8 NeuronCores.

Math notes:
  h = x @ W1.T + b1                      (fp8 on the PE, fp32 accum)
  LIF (tau=2, v_th=1, hard reset to 0):
     v' = 0.5*v + 0.5*h
     s  = (v' >= 1);  v = v' * (v' < 1)
  Scan state is kept pre-halved:  hh = 0.5*h + 0.5*b1, and per step one fused
  DVE op advances the pre-reset voltage w:
     w_new = select(w_old < 1, w_old, 0) * 0.5 + hh      (VectorE, serial)
  The not-spike mask m_t = (w_t < 1) is computed on the otherwise-idle GPSIMD
  engine, packed in pairs, and accumulated on the tensor engine with fp8
  DoubleRow identity matmuls into two half-sums (t<16 / t>=16) so the first
  half of matmul2 can run while the scan is still going.
  sum_t s_t = 16 - msum per half (exact in fp8: counts <= 16).
  y = (ssumA+ssumB) @ W2.T / T + b2;  out = log_softmax(y, axis=1).

Layout: contraction d on partitions with d = p*8 + dj so every DMA descriptor
is 2-8KB contiguous (descriptor-rate, not bandwidth, bound otherwise).
mm1 output e keeps e = j*128 + p, so W2/ssum keep the (ej p) layout.
PE is warmed with dummy matmuls during the load phase so real matmuls run at
2.4GHz from the start. The single ACT table set natural_log_exp_and_others
covers Identity/Copy/Exp/Ln so no mid-kernel table reloads occur.
"""

import numpy as np
from contextlib import ExitStack

import concourse.bass as bass
import concourse.tile as tile
from concourse import bacc, mybir
from concourse.bass_utils import run_bass_kernel_spmd

N_CORES = 8

# --- variant switches --------------------------------------------------------
ISLT_ON_GPSIMD = False   # spike mask on GPSIMD (else VectorE, baseline-style)
                         # (measured: GPSIMD software IS_LT is ~9.4us/instr - unusable)
M_FP8 = True             # spike mask in fp8 + DoubleRow paired msum matmuls
N_WARM = 9               # PE warm-up dummy matmuls (512 cols each)
REORDER_ACT_TABLES = True


def _lif_op():
    """Fused LIF step as a custom DVE op:
         out = select(in0 < s0, in0, 0) * s1 + in1
       i.e. w_new = reset(w_old)*0.5 + hh  in a single VectorE instruction."""
    from concourse import dve_ops
    from concourse.dve_spec import Spec, Src0, Src1, Zero, C0, C1, select, lower
    from concourse.dve_uop import DveOpSpec

    for op in dve_ops.OPS:
        if op.name == "LIF_STEP_ANT":
            return op
    spec = Spec(
        body=select(Src0 < C0, Src0, Zero) * C1 + Src1,
        reference=lambda in0, in1, s0, s1, imm2: (
            np.where(in0.astype(np.float32) < s0, in0.astype(np.float32), 0.0) * s1
            + in1.astype(np.float32)).astype(np.float32),
    )
    row = dve_ops._CUSTOM_DVE_ROW_BASE + len(dve_ops.OPS)
    shas = {}
    for ver in ("v3", "v4"):
        try:
            shas[ver] = DveOpSpec(name="LIF_STEP_ANT", opcode=row,
                                  uops=lower(spec, ver=ver), rd1_en=True).sha(ver)
        except Exception:
            pass
    op = dve_ops.DveOp("LIF_STEP_ANT", spec, subdim=False, uops_sha=shas)
    dve_ops.OPS.append(op)
    dve_ops._SUB_OPCODE_FOR_NAME[op.name] = row
    dve_ops.CUSTOM_DVE_SPECS[op.name] = spec
    return op


T, B, D, C = 32, 512, 1024, 1000
BC = B // N_CORES          # 64 rows per core
TB = T * BC                # 2048 matmul rows per core
NCH = 4                    # x chunks of 8 timesteps
FP32 = mybir.dt.float32
BF16 = mybir.dt.bfloat16
FP8 = mybir.dt.float8e4
W1_PRESCALE = 256.0   # host multiplies W1/W2 by this (exact power of 2) so
                      # their small uniform(-1/32,1/32) values stay in
                      # fp8e4m3's normal range; compensated on readout
AF = mybir.ActivationFunctionType
OP = mybir.AluOpType
MDT = FP8 if M_FP8 else BF16


def _prefer_combined_act_table(arch: str):
    """Force every activation we use (Identity/Copy/Exp/Ln) to resolve to the
    single set that contains them all -> zero mid-kernel ACT table reloads.
    Set order (= act_func_set_id) must stay untouched so bass's ids agree
    with the runtime act.json mapping; instead empty out the other sets."""
    from concourse.hw_specs import get_activation_tables
    t = get_activation_tables(arch)
    target = "natural_log_exp_and_others"
    if target not in t:
        return
    for k, v in t.items():
        if k != target:
            v.clear()


def build_program():
    nc = bacc.Bacc("TRN2", target_bir_lowering=False, debug=False, num_devices=N_CORES)
    if REORDER_ACT_TABLES:
        try:
            _prefer_combined_act_table(nc.m.arch)
        except Exception:
            pass

    xt_d = nc.dram_tensor("xT", [NCH * 128, 8 * 512], FP8, kind="ExternalInput").ap()
    w1t_d = nc.dram_tensor("W1T", [128, 8 * 1024], FP8, kind="ExternalInput").ap()
    b1_d = nc.dram_tensor("b1", [D], FP32, kind="ExternalInput").ap()
    w2t_d = nc.dram_tensor("W2T", [D, C], FP8, kind="ExternalInput").ap()
    b2_d = nc.dram_tensor("b2", [C], FP32, kind="ExternalInput").ap()
    y_d = nc.dram_tensor("y", [BC, C], FP32, kind="ExternalOutput").ap()

    with tile.TileContext(nc) as tc, ExitStack() as ctx:
        persist = ctx.enter_context(tc.tile_pool(name="persist", bufs=1))
        mpool = ctx.enter_context(tc.tile_pool(name="mpool", bufs=4))
        small = ctx.enter_context(tc.tile_pool(name="small", bufs=1))
        ps_h = ctx.enter_context(tc.tile_pool(name="ps_h", bufs=4, space="PSUM"))
        ps_ms = ctx.enter_context(tc.tile_pool(name="ps_ms", bufs=1, space="PSUM"))
        ps_y = ctx.enter_context(tc.tile_pool(name="ps_y", bufs=1, space="PSUM"))

        # ---- b1 first on the scalar ring (tiny, gates the h-copies) ----
        b1_sb = small.tile([128, 8], FP32)
        nc.scalar.dma_start(b1_sb[:], b1_d.rearrange("(j p) -> p j", p=128))
        b1h = small.tile([128, 8], FP32)
        nc.vector.tensor_scalar_mul(b1h[:], b1_sb[:], 0.5)

        # ---- PE warm-up: dummy matmuls run back-to-back from t=0 so the HAM
        # clock-gate opens (~3.4us) before the first real matmul arrives ----
        junk_w = small.tile([1, 2], FP8)
        nc.vector.memset(junk_w[:], 0.0)
        junk_x = small.tile([1, 512], FP8)
        nc.vector.memset(junk_x[:], 0.0)
        msumA = ps_ms.tile([128, 512], FP32, name="msumA")
        msumB = ps_ms.tile([128, 512], FP32, name="msumB")
        for i in range(N_WARM):
            nc.tensor.matmul(msumA[0:1, :], junk_w[:, 0:1], junk_x[:],
                             start=True, stop=True)

        # ---- weight / x loads. Host layouts give 2-8KB contiguous
        # descriptors. Two HWDGE rings (sync/scalar) split by partition
        # half; x2/x3 ride the vector/gpsimd rings. Order = priority. ----
        w1t = persist.tile([128, 8 * 1024], FP8)
        xt = persist.tile([128, NCH * 4096], FP8)
        w1t3 = w1t[:].rearrange("p (dj e) -> p dj e", dj=8)
        xt4 = xt[:].rearrange("p (k dj t) -> p k dj t", k=NCH, dj=8)
        xsrc = xt_d.rearrange("(k p) t -> p k t", p=128)
        xdst = xt[:].rearrange("p (k t) -> p k t", k=NCH)

        def load_w1(p0, p1, c0, c1, ring):
            ring.dma_start(w1t[p0:p1, c0:c1], w1t_d[p0:p1, c0:c1])

        def load_x(k, p0, p1, c0, c1, ring):
            ring.dma_start(xdst[p0:p1, k:k + 1, c0:c1], xsrc[p0:p1, k:k + 1, c0:c1])

        # W2T keeps the (ej p) layout (its contraction index e is fixed by
        # mm1's output layout); 1000B descriptors, queued with low priority.
        w2t = persist.tile([128, 8 * 1024], FP8)
        w2t3 = w2t[:].rearrange("p (j c) -> p j c", j=8)
        w2src = w2t_d.rearrange("(ej p) c -> p ej c", p=128)

        for (ring, p0, p1) in ((nc.sync, 0, 64), (nc.scalar, 64, 128)):
            load_w1(p0, p1, 0, 4096, ring)        # dj 0-3
            load_x(0, p0, p1, 0, 2048, ring)      # chunk0 dj 0-3
            load_w1(p0, p1, 4096, 8192, ring)     # dj 4-7
            load_x(0, p0, p1, 2048, 4096, ring)   # chunk0 dj 4-7
            load_x(1, p0, p1, 0, 4096, ring)
            load_x(2, p0, p1, 0, 4096, ring)
        nc.gpsimd.dma_start(xdst[0:64, 3:4, :], xsrc[0:64, 3:4, :])
        nc.gpsimd.dma_start(xdst[64:128, 3:4, :], xsrc[64:128, 3:4, :])
        nc.sync.dma_start(w2t3[:, 0:4, 0:C], w2src[:, 0:4, :])
        nc.scalar.dma_start(w2t3[:, 4:8, 0:C], w2src[:, 4:8, :])
        b2_sb = small.tile([1, C], FP32)
        nc.scalar.dma_start(b2_sb[:], b2_d.rearrange("(a c) -> a c", a=1))

        # ---- identity (both DoubleRow half-rows) for the spike-sum matmuls ----
        io = small.tile([128, 128], mybir.dt.int32)
        nc.gpsimd.iota(io[:], pattern=[[1, 128]], base=0, channel_multiplier=-1)
        ident2 = small.tile([128, 256], MDT)
        nc.vector.tensor_scalar(ident2[:, 0:128], io[:], 0, None, op0=OP.is_equal)
        nc.vector.tensor_scalar(ident2[:, 128:256], io[:], 0, None, op0=OP.is_equal)
        ident2v = ident2[:].rearrange("p (r q) -> p r q", r=2)

        # ---- matmul1: h[e, tb] = W1 @ x.T, fused 0.5*h + 0.5*b1 into scan
        # layout. h_sb free index = t*512 + j*64 + b ----
        h_sb = persist.tile([128, T * 512], BF16)
        h3 = h_sb[:].rearrange("p (t x) -> p t x", x=512)

        def mm1_group(g):
            t0 = 8 * g
            for j in range(8):
                ps = ps_h.tile([128, 512], FP32, tag="ps_h", name=f"psh_{g}_{j}")
                for dp in range(4):   # pairs of contraction tiles (DoubleRow)
                    nc.tensor.matmul(
                        ps[:],
                        w1t3[:, 2 * dp:2 * dp + 2, j * 128:(j + 1) * 128],
                        xt4[:, g, 2 * dp:2 * dp + 2, :],
                        start=(dp == 0), stop=(dp == 3),
                        perf_mode=mybir.MatmulPerfMode.DoubleRow,
                    )
                nc.scalar.activation(
                    h3[:, t0:t0 + 8, j * 64:(j + 1) * 64],
                    ps[:].rearrange("p (t b) -> p t b", t=8),
                    AF.Identity, scale=0.5 / W1_PRESCALE, bias=b1h[:, j:j + 1],
                )

        # ---- LIF scan pieces (emitted interleaved with mm1 groups) ----
        lif = _lif_op()
        wst = [small.tile([128, 512], BF16, name=f"wst{i}") for i in range(2)]
        nc.vector.memset(wst[0][:], 0.0)
        m_tiles = {}

        def scan_steps(t0, t1):
            for t in range(t0, t1):
                prev, cur = wst[t % 2], wst[(t + 1) % 2]
                if t == 0:
                    prev = wst[0]
                nc.vector._custom_dve(lif, out=cur[:], in0=prev[:],
                                      in1=h3[:, t, :], s0=1.0, s1=0.5)
                if t % 2 == 0:
                    m_tiles[t // 2] = mpool.tile([128, 1024], MDT, tag="m",
                                                 name=f"m{t // 2}")
                m2 = m_tiles[t // 2]
                dst = m2[:, 0:512] if t % 2 == 0 else m2[:, 512:1024]
                eng = nc.gpsimd if ISLT_ON_GPSIMD else nc.vector
                eng.tensor_scalar(dst, cur[:], 1.0, None, op0=OP.is_lt)

        def msum_pairs(i0, i1):
            # msum += m_{2i} + m_{2i+1} via fp8 DoubleRow identity matmul
            for i in range(i0, i1):
                tgt = msumA if i < 8 else msumB
                m2v = m_tiles[i][:].rearrange("p (r x) -> p r x", r=2)
                if M_FP8:
                    nc.tensor.matmul(tgt[:], ident2v, m2v,
                                     start=(i in (0, 8)), stop=(i in (7, 15)),
                                     perf_mode=mybir.MatmulPerfMode.DoubleRow)
                else:
                    for r in range(2):
                        nc.tensor.matmul(tgt[:], ident2v[:, 0:1, :].rearrange("p r q -> p (r q)"),
                                         m2v[:, r:r + 1, :].rearrange("p r x -> p (r x)"),
                                         start=(i in (0, 8) and r == 0),
                                         stop=(i in (7, 15) and r == 1))

        # epilogue constants (ACT ops emitted early so they sit before the
        # h-copies in the ACT queue and run during the load phase)
        b2_32 = small.tile([1, C], BF16)
        nc.scalar.activation(b2_32[:], b2_sb[:], AF.Copy,
                             scale=float(T) * W1_PRESCALE)
        ones = small.tile([1, BC], BF16)
        nc.vector.memset(ones[:], 1.0)

        # ---- emission: mm1 groups + interleaved scan / spike sums / mm2a ----
        mm1_group(0)
        mm1_group(1)
        scan_steps(0, 8)
        msum_pairs(0, 4)
        mm1_group(2)
        scan_steps(8, 16)
        msum_pairs(4, 8)           # completes msumA (t 0..15)

        ssumA = small.tile([128, 512], FP8)
        nc.scalar.activation(ssumA[:], msumA[:], AF.Copy, scale=-1.0, bias=16.0)
        ssumA3 = ssumA[:].rearrange("p (j b) -> p j b", j=8)

        y_sb = small.tile([BC, 1024], FP32)
        psy = [ps_y.tile([BC, 512], FP32, tag="ps_y", name=f"psy{h}")
               for h in range(2)]

        def mm2(ssum3, first, last):
            for half in range(2):
                n = 512 if half == 0 else C - 512
                c0 = half * 512
                for pj in range(4):   # DoubleRow pairs of e-tiles
                    nc.tensor.matmul(
                        psy[half][:, 0:n],
                        ssum3[:, 2 * pj:2 * pj + 2, :],
                        w2t3[:, 2 * pj:2 * pj + 2, c0:c0 + n],
                        start=(first and pj == 0),
                        stop=(last and pj == 3),
                        perf_mode=mybir.MatmulPerfMode.DoubleRow,
                    )
                if first:   # bias rides in the first accumulation pass only
                    nc.tensor.matmul(psy[half][:, 0:n], ones[:],
                                     b2_32[:, c0:c0 + n], start=False, stop=False)

        mm2(ssumA3, True, False)   # runs while the scan continues
        mm1_group(3)
        scan_steps(16, 24)
        msum_pairs(8, 12)
        scan_steps(24, 32)
        msum_pairs(12, 16)         # completes msumB (t 16..31)

        ssumB = small.tile([128, 512], FP8)
        nc.scalar.activation(ssumB[:], msumB[:], AF.Copy, scale=-1.0, bias=16.0)
        ssumB3 = ssumB[:].rearrange("p (j b) -> p j b", j=8)
        mm2(ssumB3, False, True)

        # ---- log_softmax over C. |y| <= ~35 so no max-shift needed; the
        # Exp's free-dim accumulator replaces a separate reduce. ----
        ez = small.tile([BC, 1024], BF16)
        esum = small.tile([BC, 2], FP32)
        esum1 = small.tile([BC, 1], FP32)
        lse = small.tile([BC, 1], FP32)
        out_sb = small.tile([BC, C], FP32)
        for half in range(2):
            n = 512 if half == 0 else C - 512
            c0 = half * 512
            nc.scalar.activation(y_sb[:, c0:c0 + n], psy[half][:, 0:n], AF.Copy,
                                 scale=1.0 / (T * W1_PRESCALE))
            nc.scalar.activation(ez[:, c0:c0 + n], y_sb[:, c0:c0 + n], AF.Exp,
                                 accum_out=esum[:, half:half + 1])
        nc.vector.tensor_tensor(esum1[:], esum[:, 0:1], esum[:, 1:2], op=OP.add)
        nc.scalar.activation(lse[:], esum1[:], AF.Ln)
        for half, ring in ((0, nc.sync), (1, nc.scalar)):
            n = 512 if half == 0 else C - 512
            c0 = half * 512
            nc.vector.tensor_scalar(out_sb[:, c0:c0 + n], y_sb[:, c0:c0 + n],
                                    lse[:], None, op0=OP.subtract)
            ring.dma_start(y_d[:, c0:c0 + n], out_sb[:, c0:c0 + n])

    nc.compile()
    return nc


_CACHE = {}


def kernel(x, W1, b1, W2, b2):
    if "nc" not in _CACHE:
        _CACHE["nc"] = build_program()
    nc = _CACHE["nc"]

    f8 = mybir.dt.np(FP8)
    x = np.asarray(x, dtype=np.float32)
    w1t = np.ascontiguousarray(
        (np.asarray(W1, dtype=np.float32).T * W1_PRESCALE).astype(f8)
    ).reshape(128, 8 * 1024)          # d = p*8+dj on partitions
    w2t = np.ascontiguousarray(
        (np.asarray(W2, dtype=np.float32).T * W1_PRESCALE).astype(f8))
    b1 = np.ascontiguousarray(b1, dtype=np.float32)
    b2 = np.ascontiguousarray(b2, dtype=np.float32)
    in_maps = []
    for i in range(N_CORES):
        xs = x[:, i * BC:(i + 1) * BC, :].reshape(TB, D).T.astype(f8)
        # [d, tb] -> [k, p, dj, t'] with d = p*8+dj, tb = k*512+t'
        xs4 = xs.reshape(128, 8, NCH, 512).transpose(2, 0, 1, 3)
        xhost = np.ascontiguousarray(xs4).reshape(NCH * 128, 8 * 512)
        in_maps.append({"xT": xhost, "W1T": w1t, "b1": b1, "W2T": w2t, "b2": b2})

    res = run_bass_kernel_spmd(nc, in_maps, core_ids=list(range(N_CORES)),
                               **_CACHE.get("run_kwargs", {}))
    _CACHE["last_results"] = res
    out = np.concatenate([res.results[i]["y"] for i in range(N_CORES)], axis=0)
    return out


# revision 14
# speedup vs baseline: 1.0559x; 1.0559x over previous
"""Trainium2 Bass kernel for nn_CLFBlock (linear -> LIF scan -> linear -> T-mean -> log_softmax).

Self-contained: hardcodes shapes T=32, B=512, D=1024, C=1000 and data-parallel
sharding of the batch dim across 8 NeuronCores.

Math notes:
  h = x @ W1.T + b1                      (fp8 on the PE, fp32 accum)
  LIF (tau=2, v_th=1, hard reset to 0):
     v' = 0.5*v + 0.5*h
     s  = (v' >= 1);  v = v' * (v' < 1)
  Scan state is kept pre-halved:  hh = 0.5*h + 0.5*b1, and per step one fused
  DVE op advances the pre-reset voltage w:
     w_new = select(w_old < 1, w_old, 0) * 0.5 + hh      (VectorE, serial)
  The not-spike mask m_t = (w_t < 1) is computed on the otherwise-idle GPSIMD
  engine, packed in pairs, and accumulated on the tensor engine with fp8
  DoubleRow identity matmuls into two half-sums (t<16 / t>=16) so the first
  half of matmul2 can run while the scan is still going.
  sum_t s_t = 16 - msum per half (exact in fp8: counts <= 16).
  y = (ssumA+ssumB) @ W2.T / T + b2;  out = log_softmax(y, axis=1).

Layout: contraction d on partitions with d = p*8 + dj so every DMA descriptor
is 2-8KB contiguous (descriptor-rate, not bandwidth, bound otherwise).
mm1 output e keeps e = j*128 + p, so W2/ssum keep the (ej p) layout.
PE is warmed with dummy matmuls during the load phase so real matmuls run at
2.4GHz from the start. The single ACT table set natural_log_exp_and_others
covers Identity/Copy/Exp/Ln so no mid-kernel table reloads occur.
"""

import numpy as np
from contextlib import ExitStack

import concourse.bass as bass
import concourse.tile as tile
from concourse import bacc, mybir
from concourse.bass_utils import run_bass_kernel_spmd

N_CORES = 8

# --- variant switches --------------------------------------------------------
ISLT_ON_GPSIMD = False   # spike mask on GPSIMD (else VectorE, baseline-style)
                         # (measured: GPSIMD software IS_LT is ~9.4us/instr - unusable)
M_FP8 = True             # spike mask in fp8 + DoubleRow paired msum matmuls
N_WARM = 28              # PE warm-up dummy matmuls (512 cols each)
REORDER_ACT_TABLES = True


def _lif_op():
    """Fused LIF step as a custom DVE op:
         out = select(in0 < s0, in0, 0) * s1 + in1
       i.e. w_new = reset(w_old)*0.5 + hh  in a single VectorE instruction."""
    from concourse import dve_ops
    from concourse.dve_spec import Spec, Src0, Src1, Zero, C0, C1, select, lower
    from concourse.dve_uop import DveOpSpec

    for op in dve_ops.OPS:
        if op.name == "LIF_STEP_ANT":
            return op
    spec = Spec(
        body=select(Src0 < C0, Src0, Zero) * C1 + Src1,
        reference=lambda in0, in1, s0, s1, imm2: (
            np.where(in0.astype(np.float32) < s0, in0.astype(np.float32), 0.0) * s1
            + in1.astype(np.float32)).astype(np.float32),
    )
    row = dve_ops._CUSTOM_DVE_ROW_BASE + len(dve_ops.OPS)
    shas = {}
    for ver in ("v3", "v4"):
        try:
            shas[ver] = DveOpSpec(name="LIF_STEP_ANT", opcode=row,
                                  uops=lower(spec, ver=ver), rd1_en=True).sha(ver)
        except Exception:
            pass
    op = dve_ops.DveOp("LIF_STEP_ANT", spec, subdim=False, uops_sha=shas)
    dve_ops.OPS.append(op)
    dve_ops._SUB_OPCODE_FOR_NAME[op.name] = row
    dve_ops.CUSTOM_DVE_SPECS[op.name] = spec
    return op


T, B, D, C = 32, 512, 1024, 1000
BC = B // N_CORES          # 64 rows per core
TB = T * BC                # 2048 matmul rows per core
NCH = 4                    # x chunks of 8 timesteps
FP32 = mybir.dt.float32
BF16 = mybir.dt.bfloat16
FP8 = mybir.dt.float8e4
W1_PRESCALE = 256.0   # host multiplies W1/W2 by this (exact power of 2) so
                      # their small uniform(-1/32,1/32) values stay in
                      # fp8e4m3's normal range; compensated on readout
AF = mybir.ActivationFunctionType
OP = mybir.AluOpType
MDT = FP8 if M_FP8 else BF16


def _prefer_combined_act_table(arch: str):
    """Force every activation we use (Identity/Copy/Exp/Ln) to resolve to the
    single set that contains them all -> zero mid-kernel ACT table reloads.
    Set order (= act_func_set_id) must stay untouched so bass's ids agree
    with the runtime act.json mapping; instead empty out the other sets."""
    from concourse.hw_specs import get_activation_tables
    t = get_activation_tables(arch)
    target = "natural_log_exp_and_others"
    if target not in t:
        return
    for k, v in t.items():
        if k != target:
            v.clear()


def build_program():
    nc = bacc.Bacc("TRN2", target_bir_lowering=False, debug=False, num_devices=N_CORES)
    if REORDER_ACT_TABLES:
        try:
            _prefer_combined_act_table(nc.m.arch)
        except Exception:
            pass

    xt_d = nc.dram_tensor("xT", [NCH * 128, 8 * 512], FP8, kind="ExternalInput").ap()
    w1t_d = nc.dram_tensor("W1T", [128, 8 * 1024], FP8, kind="ExternalInput").ap()
    b1_d = nc.dram_tensor("b1", [D], FP32, kind="ExternalInput").ap()
    w2t_d = nc.dram_tensor("W2T", [D, C], FP8, kind="ExternalInput").ap()
    b2_d = nc.dram_tensor("b2", [C], FP32, kind="ExternalInput").ap()
    y_d = nc.dram_tensor("y", [BC, C], FP32, kind="ExternalOutput").ap()

    with tile.TileContext(nc) as tc, ExitStack() as ctx:
        persist = ctx.enter_context(tc.tile_pool(name="persist", bufs=1))
        mpool = ctx.enter_context(tc.tile_pool(name="mpool", bufs=4))
        small = ctx.enter_context(tc.tile_pool(name="small", bufs=1))
        ps_h = ctx.enter_context(tc.tile_pool(name="ps_h", bufs=4, space="PSUM"))
        ps_ms = ctx.enter_context(tc.tile_pool(name="ps_ms", bufs=1, space="PSUM"))
        ps_y = ctx.enter_context(tc.tile_pool(name="ps_y", bufs=1, space="PSUM"))

        # ---- b1 first on the scalar ring (tiny, gates the h-copies) ----
        b1_sb = small.tile([128, 8], FP32)
        nc.scalar.dma_start(b1_sb[:], b1_d.rearrange("(j p) -> p j", p=128))
        b1h = small.tile([128, 8], FP32)
        nc.vector.tensor_scalar_mul(b1h[:], b1_sb[:], 0.5)

        # ---- PE warm-up: dummy matmuls run back-to-back from t=0 so the HAM
        # clock-gate opens (~3.4us) before the first real matmul arrives ----
        junk_w = small.tile([1, 2], FP8)
        nc.vector.memset(junk_w[:], 0.0)
        junk_x = small.tile([1, 512], FP8)
        nc.vector.memset(junk_x[:], 0.0)
        msumA = ps_ms.tile([128, 512], FP32, name="msumA")
        msumB = ps_ms.tile([128, 512], FP32, name="msumB")
        for i in range(N_WARM):
            nc.tensor.matmul(msumA[0:1, :], junk_w[:, 0:1], junk_x[:],
                             start=True, stop=True)

        # ---- weight / x loads. Host layouts give 2-8KB contiguous
        # descriptors. Two HWDGE rings (sync/scalar) split by partition
        # half; x2/x3 ride the vector/gpsimd rings. Order = priority. ----
        w1t = persist.tile([128, 8 * 1024], FP8)
        xt = persist.tile([128, NCH * 4096], FP8)
        w1t3 = w1t[:].rearrange("p (dj e) -> p dj e", dj=8)
        xt4 = xt[:].rearrange("p (k dj t) -> p k dj t", k=NCH, dj=8)
        xsrc = xt_d.rearrange("(k p) t -> p k t", p=128)
        xdst = xt[:].rearrange("p (k t) -> p k t", k=NCH)

        def load_w1(p0, p1, c0, c1, ring):
            ring.dma_start(w1t[p0:p1, c0:c1], w1t_d[p0:p1, c0:c1])

        def load_x(k, p0, p1, c0, c1, ring):
            ring.dma_start(xdst[p0:p1, k:k + 1, c0:c1], xsrc[p0:p1, k:k + 1, c0:c1])

        # W2T keeps the (ej p) layout (its contraction index e is fixed by
        # mm1's output layout); 1000B descriptors, queued with low priority.
        w2t = persist.tile([128, 8 * 1024], FP8)
        w2t3 = w2t[:].rearrange("p (j c) -> p j c", j=8)
        w2src = w2t_d.rearrange("(ej p) c -> p ej c", p=128)

        b2_sb = small.tile([1, C], FP32)
        nc.scalar.dma_start(b2_sb[:], b2_d.rearrange("(a c) -> a c", a=1))
        load_w1(0, 128, 0, 4096, nc.sync)         # dj 0-3
        load_x(0, 0, 128, 0, 2048, nc.sync)       # chunk0 dj 0-3
        load_w1(0, 128, 4096, 8192, nc.sync)      # dj 4-7
        load_x(0, 0, 128, 2048, 4096, nc.sync)    # chunk0 dj 4-7
        load_x(1, 0, 128, 0, 4096, nc.sync)
        load_x(2, 0, 128, 0, 4096, nc.sync)
        load_x(3, 0, 128, 0, 4096, nc.sync)
        nc.sync.dma_start(w2t3[:, 0:4, 0:C], w2src[:, 0:4, :])
        nc.sync.dma_start(w2t3[:, 4:8, 0:C], w2src[:, 4:8, :])

        # ---- identity (both DoubleRow half-rows) for the spike-sum matmuls ----
        io = small.tile([128, 128], mybir.dt.int32)
        nc.gpsimd.iota(io[:], pattern=[[1, 128]], base=0, channel_multiplier=-1)
        identM = small.tile([128, 256], MDT)   # value -1 on the diagonal
        identS = small.tile([128, 256], MDT)   # value +0.5 on the diagonal
        for h in range(2):
            nc.vector.tensor_scalar(identM[:, 128 * h:128 * h + 128], io[:],
                                    0, None, op0=OP.is_equal)
            nc.vector.tensor_scalar(identS[:, 128 * h:128 * h + 128], io[:],
                                    0, None, op0=OP.is_equal)
        nc.vector.tensor_scalar_mul(identM[:], identM[:], -1.0)
        nc.vector.tensor_scalar_mul(identS[:], identS[:], 0.5)
        identMv = identM[:].rearrange("p (r q) -> p r q", r=2)
        identSv = identS[:].rearrange("p (r q) -> p r q", r=2)
        negone = small.tile([128, 1], FP32)
        nc.vector.memset(negone[:], -1.0)

        # ---- matmul1: h[e, tb] = W1 @ x.T, fused 0.5*h + 0.5*b1 into scan
        # layout. h_sb free index = t*512 + j*64 + b ----
        h_sb = persist.tile([128, T * 512], BF16)
        h3 = h_sb[:].rearrange("p (t x) -> p t x", x=512)

        def mm1_group(g):
            t0 = 8 * g
            for j in range(8):
                ps = ps_h.tile([128, 512], FP32, tag="ps_h", name=f"psh_{g}_{j}")
                for dp in range(4):   # pairs of contraction tiles (DoubleRow)
                    nc.tensor.matmul(
                        ps[:],
                        w1t3[:, 2 * dp:2 * dp + 2, j * 128:(j + 1) * 128],
                        xt4[:, g, 2 * dp:2 * dp + 2, :],
                        start=(dp == 0), stop=(dp == 3),
                        perf_mode=mybir.MatmulPerfMode.DoubleRow,
                    )
                nc.scalar.activation(
                    h3[:, t0:t0 + 8, j * 64:(j + 1) * 64],
                    ps[:].rearrange("p (t b) -> p t b", t=8),
                    AF.Identity, scale=0.5 / W1_PRESCALE, bias=b1h[:, j:j + 1],
                )

        # ---- LIF scan pieces (emitted interleaved with mm1 groups) ----
        lif = _lif_op()
        wst = [small.tile([128, 512], BF16, name=f"wst{i}") for i in range(2)]
        nc.vector.memset(wst[0][:], 0.0)
        m_tiles = {}

        def scan_steps(t0, t1):
            for t in range(t0, t1):
                prev, cur = wst[t % 2], wst[(t + 1) % 2]
                if t == 0:
                    prev = wst[0]
                nc.vector._custom_dve(lif, out=cur[:], in0=prev[:],
                                      in1=h3[:, t, :], s0=1.0, s1=0.5)
                if t % 2 == 0:
                    m_tiles[t // 2] = mpool.tile([128, 1024], MDT, tag="m",
                                                 name=f"m{t // 2}")
                m2 = m_tiles[t // 2]
                dst = m2[:, 0:512] if t % 2 == 0 else m2[:, 512:1024]
                if t < 24:
                    nc.vector.tensor_scalar(dst, cur[:], 1.0, None, op0=OP.is_lt)
                else:
                    # ACT is idle by now; sign(w-1) = 2s-1 rides the same
                    # accumulation with a +0.5-scaled identity
                    nc.scalar.activation(dst, cur[:], AF.Sign, bias=negone[:, 0:1])

        def msum_pairs(i0, i1):
            # msumA_raw = -sum_{t<16} m_t            = ssumA - 16
            # msumB_raw = -sum_{16..23} m_t + 0.5*sum_{24..31} sign(w-1)
            #           = (sA-8) + (sB-4)            = ssumB - 12
            for i in range(i0, i1):
                tgt = msumA if i < 8 else msumB
                idv = identMv if i < 12 else identSv
                m2v = m_tiles[i][:].rearrange("p (r x) -> p r x", r=2)
                nc.tensor.matmul(tgt[:], idv, m2v,
                                 start=(i in (0, 8)), stop=(i in (7, 15)),
                                 perf_mode=mybir.MatmulPerfMode.DoubleRow)

        ones = small.tile([1, BC], BF16)
        nc.vector.memset(ones[:], 1.0)

        # ---- emission: mm1 groups + interleaved scan / spike sums; mm2a
        # goes after g3 so it fills the PE while the scan tail runs ----
        mm1_group(0)
        mm1_group(1)
        scan_steps(0, 8)
        msum_pairs(0, 4)
        mm1_group(2)
        scan_steps(8, 16)
        msum_pairs(4, 8)           # completes msumA (t 0..15)
        mm1_group(3)

        b2_32 = small.tile([1, C], BF16)
        nc.scalar.activation(b2_32[:], b2_sb[:], AF.Copy,
                             scale=float(T) * W1_PRESCALE)
        ssumA = small.tile([128, 512], FP8)
        nc.scalar.activation(ssumA[:], msumA[:], AF.Copy, scale=1.0, bias=16.0)
        ssumA3 = ssumA[:].rearrange("p (j b) -> p j b", j=8)

        y_sb = small.tile([BC, 1024], FP32)
        psy = [ps_y.tile([BC, 512], FP32, tag="ps_y", name=f"psy{h}")
               for h in range(2)]

        def mm2(ssum3, first, last):
            for half in range(2):
                n = 512 if half == 0 else C - 512
                c0 = half * 512
                for pj in range(4):   # DoubleRow pairs of e-tiles
                    nc.tensor.matmul(
                        psy[half][:, 0:n],
                        ssum3[:, 2 * pj:2 * pj + 2, :],
                        w2t3[:, 2 * pj:2 * pj + 2, c0:c0 + n],
                        start=(first and pj == 0),
                        stop=(last and pj == 3),
                        perf_mode=mybir.MatmulPerfMode.DoubleRow,
                    )
                if first:   # bias rides in the first accumulation pass only
                    nc.tensor.matmul(psy[half][:, 0:n], ones[:],
                                     b2_32[:, c0:c0 + n], start=False, stop=False)

        mm2(ssumA3, True, False)   # runs while the scan continues
        scan_steps(16, 24)
        msum_pairs(8, 12)
        scan_steps(24, 32)
        msum_pairs(12, 16)         # completes msumB (t 16..31)

        ssumB = small.tile([128, 512], FP8)
        nc.scalar.activation(ssumB[:], msumB[:], AF.Copy, scale=1.0, bias=12.0)
        ssumB3 = ssumB[:].rearrange("p (j b) -> p j b", j=8)
        mm2(ssumB3, False, True)

        # ---- log_softmax over C. |y| <= ~35 so no max-shift needed; the
        # Exp's free-dim accumulator replaces a separate reduce. ----
        ez = small.tile([BC, 1024], BF16)
        esum = small.tile([BC, 2], FP32)
        esum1 = small.tile([BC, 1], FP32)
        lse = small.tile([BC, 1], FP32)
        out_sb = small.tile([BC, C], FP32)
        for half in range(2):
            n = 512 if half == 0 else C - 512
            c0 = half * 512
            nc.scalar.activation(y_sb[:, c0:c0 + n], psy[half][:, 0:n], AF.Copy,
                                 scale=1.0 / (T * W1_PRESCALE))
            nc.scalar.activation(ez[:, c0:c0 + n], y_sb[:, c0:c0 + n], AF.Exp,
                                 accum_out=esum[:, half:half + 1])
        nc.vector.tensor_tensor(esum1[:], esum[:, 0:1], esum[:, 1:2], op=OP.add)
        nc.scalar.activation(lse[:], esum1[:], AF.Ln)
        for half, ring in ((0, nc.sync), (1, nc.scalar)):
            n = 512 if half == 0 else C - 512
            c0 = half * 512
            nc.vector.tensor_scalar(out_sb[:, c0:c0 + n], y_sb[:, c0:c0 + n],
                                    lse[:], None, op0=OP.subtract)
            ring.dma_start(y_d[:, c0:c0 + n], out_sb[:, c0:c0 + n])

    nc.compile()
    return nc


_CACHE = {}


def kernel(x, W1, b1, W2, b2):
    if "nc" not in _CACHE:
        _CACHE["nc"] = build_program()
    nc = _CACHE["nc"]

    f8 = mybir.dt.np(FP8)
    x = np.asarray(x, dtype=np.float32)
    w1t = np.ascontiguousarray(
        (np.asarray(W1, dtype=np.float32).T * W1_PRESCALE).astype(f8)
    ).reshape(128, 8 * 1024)          # d = p*8+dj on partitions
    w2t = np.ascontiguousarray(
        (np.asarray(W2, dtype=np.float32).T * W1_PRESCALE).astype(f8))
    b1 = np.ascontiguousarray(b1, dtype=np.float32)
    b2 = np.ascontiguousarray(b2, dtype=np.float32)
    in_maps = []
    for i in range(N_CORES):
        xs = x[:, i * BC:(i + 1) * BC, :].reshape(TB, D).T.astype(f8)
        # [d, tb] -> [k, p, dj, t'] with d = p*8+dj, tb = k*512+t'
        xs4 = xs.reshape(128, 8, NCH, 512).transpose(2, 0, 1, 3)
        xhost = np.ascontiguousarray(xs4).reshape(NCH * 128, 8 * 512)
        in_maps.append({"xT": xhost, "W1T": w1t, "b1": b1, "W2T": w2t, "b2": b2})

    res = run_bass_kernel_spmd(nc, in_maps, core_ids=list(range(N_CORES)),
                               **_CACHE.get("run_kwargs", {}))
    _CACHE["last_results"] = res
    out = np.concatenate([res.results[i]["y"] for i in range(N_CORES)], axis=0)
    return out


# revision 15
# speedup vs baseline: 1.0859x; 1.0284x over previous
"""Trainium2 Bass kernel for nn_CLFBlock (linear -> LIF scan -> linear -> T-mean -> log_softmax).

Self-contained: hardcodes shapes T=32, B=512, D=1024, C=1000 and data-parallel
sharding of the batch dim across 8 NeuronCores.

Math notes:
  h = x @ W1.T + b1                      (fp8 on the PE, fp32 accum)
  LIF (tau=2, v_th=1, hard reset to 0):
     v' = 0.5*v + 0.5*h
     s  = (v' >= 1);  v = v' * (v' < 1)
  Scan state is kept pre-halved:  hh = 0.5*h + 0.5*b1, and per step one fused
  DVE op advances the pre-reset voltage w:
     w_new = select(w_old < 1, w_old, 0) * 0.5 + hh      (VectorE, serial)
  The spike masks are accumulated on the tensor engine with scaled identity
  matmuls into two half-sums so the first half of matmul2 can run while the
  scan is still going:
    t<24:  m = (w<1) on VectorE (bf16), identity value -1
    t>=24: m = sign(w-1) on the by-then-idle ScalarE, identity value +0.5
           (also keeps PE duty-cycle up through the scan tail so the HAM
            clock-gate stays open for matmul2)
  msumA_raw = -sum_{t<16} m  = ssumA - 16          -> ssumA = raw + 16
  msumB_raw = -sum_{16..23} m + 0.5*sum_{24..31} sign = ssumB - 12
  ssum halves are exact in fp8 (counts <= 16).
  y = (ssumA+ssumB) @ W2.T / T + b2;  out = log_softmax(y, axis=1)
  (no max-shift needed: |y| small; Exp's free-dim accumulator replaces the
  reduce, and a single ACT table set covers Identity/Copy/Exp/Ln/Sign so no
  mid-kernel table reloads occur).

Layout: mm1 contraction d sits on partitions as d = p*8 + dj so every load
descriptor is 4-12KB contiguous (small descriptors are descriptor-rate
bound at ~12GB/s/queue). mm1 output keeps e = j*128 + p, so W2/ssum keep
the (ej p) layout. The PE is warmed with full-array dummy matmuls during
the load phase (1x1 dummies are invisible to the HAM activity monitor).
"""

import numpy as np
from contextlib import ExitStack

import concourse.bass as bass
import concourse.tile as tile
from concourse import bacc, mybir
from concourse.bass_utils import run_bass_kernel_spmd

N_CORES = 8
N_WARM = 26              # PE warm-up dummy matmuls (full array, 512 cols)


def _lif_op():
    """Fused LIF step as a custom DVE op:
         out = select(in0 < s0, in0, 0) * s1 + in1
       i.e. w_new = reset(w_old)*0.5 + hh  in a single VectorE instruction."""
    from concourse import dve_ops
    from concourse.dve_spec import Spec, Src0, Src1, Zero, C0, C1, select, lower
    from concourse.dve_uop import DveOpSpec

    for op in dve_ops.OPS:
        if op.name == "LIF_STEP_ANT":
            return op
    spec = Spec(
        body=select(Src0 < C0, Src0, Zero) * C1 + Src1,
        reference=lambda in0, in1, s0, s1, imm2: (
            np.where(in0.astype(np.float32) < s0, in0.astype(np.float32), 0.0) * s1
            + in1.astype(np.float32)).astype(np.float32),
    )
    row = dve_ops._CUSTOM_DVE_ROW_BASE + len(dve_ops.OPS)
    shas = {}
    for ver in ("v3", "v4"):
        try:
            shas[ver] = DveOpSpec(name="LIF_STEP_ANT", opcode=row,
                                  uops=lower(spec, ver=ver), rd1_en=True).sha(ver)
        except Exception:
            pass
    op = dve_ops.DveOp("LIF_STEP_ANT", spec, subdim=False, uops_sha=shas)
    dve_ops.OPS.append(op)
    dve_ops._SUB_OPCODE_FOR_NAME[op.name] = row
    dve_ops.CUSTOM_DVE_SPECS[op.name] = spec
    return op


T, B, D, C = 32, 512, 1024, 1000
BC = B // N_CORES          # 64 rows per core
TB = T * BC                # 2048 matmul rows per core
NCH = 4                    # x chunks of 8 timesteps
FP32 = mybir.dt.float32
BF16 = mybir.dt.bfloat16
FP8 = mybir.dt.float8e4
W1_PRESCALE = 256.0   # host multiplies W1/W2 by this (exact power of 2) so
                      # their small uniform(-1/32,1/32) values stay in
                      # fp8e4m3's normal range; compensated on readout
AF = mybir.ActivationFunctionType
OP = mybir.AluOpType


def _prefer_combined_act_table(arch: str):
    """Force every activation we use (Identity/Copy/Exp/Ln/Sign) to resolve
    to the single set containing them all -> zero mid-kernel table reloads.
    Set order (= act_func_set_id) must stay untouched so bass's ids agree
    with the runtime act.json mapping; instead empty out the other sets."""
    from concourse.hw_specs import get_activation_tables
    t = get_activation_tables(arch)
    target = "natural_log_exp_and_others"
    if target not in t:
        return
    for k, v in t.items():
        if k != target:
            v.clear()


def build_program():
    nc = bacc.Bacc("TRN2", target_bir_lowering=False, debug=False, num_devices=N_CORES)
    try:
        _prefer_combined_act_table(nc.m.arch)
    except Exception:
        pass

    xt_d = nc.dram_tensor("xT", [128, NCH * 4096], FP8, kind="ExternalInput").ap()
    w1t_d = nc.dram_tensor("W1T", [128, 8 * 1024], FP8, kind="ExternalInput").ap()
    b1_d = nc.dram_tensor("b1", [D], FP32, kind="ExternalInput").ap()
    w2t_d = nc.dram_tensor("W2T", [D, C], FP8, kind="ExternalInput").ap()
    b2_d = nc.dram_tensor("b2", [C], FP32, kind="ExternalInput").ap()
    y_d = nc.dram_tensor("y", [BC, C], FP32, kind="ExternalOutput").ap()

    with tile.TileContext(nc) as tc, ExitStack() as ctx:
        persist = ctx.enter_context(tc.tile_pool(name="persist", bufs=1))
        mpool = ctx.enter_context(tc.tile_pool(name="mpool", bufs=4))
        small = ctx.enter_context(tc.tile_pool(name="small", bufs=1))
        ps_h = ctx.enter_context(tc.tile_pool(name="ps_h", bufs=4, space="PSUM"))
        ps_ms = ctx.enter_context(tc.tile_pool(name="ps_ms", bufs=1, space="PSUM"))
        ps_y = ctx.enter_context(tc.tile_pool(name="ps_y", bufs=1, space="PSUM"))

        # ---- tiny loads on the scalar ring (b1 gates the h-copies) ----
        b1_sb = small.tile([128, 8], FP32)
        nc.scalar.dma_start(b1_sb[:], b1_d.rearrange("(j p) -> p j", p=128))
        b2_sb = small.tile([1, C], FP32)
        nc.scalar.dma_start(b2_sb[:], b2_d.rearrange("(a c) -> a c", a=1))
        b1h = small.tile([128, 8], FP32)
        nc.vector.tensor_scalar_mul(b1h[:], b1_sb[:], 0.5)

        # ---- all big loads on the (otherwise idle) sync ring, in strict
        # priority order; descriptors are 4-12KB contiguous per partition ----
        w1t = persist.tile([128, 8 * 1024], FP8)
        xt = persist.tile([128, NCH * 4096], FP8)
        w2t = persist.tile([128, 8 * 1024], FP8)
        w1t3 = w1t[:].rearrange("p (dj e) -> p dj e", dj=8)
        xt4 = xt[:].rearrange("p (k dj t) -> p k dj t", k=NCH, dj=8)
        w2t3 = w2t[:].rearrange("p (j c) -> p j c", j=8)
        w2src = w2t_d.rearrange("(ej p) c -> p ej c", p=128)

        nc.sync.dma_start(w1t[:], w1t_d[:])
        nc.sync.dma_start(xt[:, 0:4096], xt_d[:, 0:4096])            # chunk 0
        nc.sync.dma_start(xt[:, 4096:16384], xt_d[:, 4096:16384])    # chunks 1-3
        nc.sync.dma_start(w2t3[:, 0:4, 0:C], w2src[:, 0:4, :])
        nc.sync.dma_start(w2t3[:, 4:8, 0:C], w2src[:, 4:8, :])

        # ---- PE warm-up: full-array dummy matmuls run back-to-back from t=0
        # so the HAM clock-gate opens before the real matmuls arrive ----
        junk_w = small.tile([128, 128], FP8)
        nc.vector.memset(junk_w[:], 0.0)
        junk_x = small.tile([128, 512], FP8)
        nc.vector.memset(junk_x[:], 0.0)
        msumA = ps_ms.tile([128, 512], FP32, name="msumA")
        msumB = ps_ms.tile([128, 512], FP32, name="msumB")
        for i in range(N_WARM):
            nc.tensor.matmul(msumA[:], junk_w[:], junk_x[:], start=True, stop=True)

        # ---- scaled identities for the spike-sum matmuls ----
        io = small.tile([128, 128], mybir.dt.int32)
        nc.gpsimd.iota(io[:], pattern=[[1, 128]], base=0, channel_multiplier=-1)
        identM = small.tile([128, 128], BF16)   # -1 on the diagonal
        identS = small.tile([128, 128], BF16)   # +0.5 on the diagonal
        nc.vector.tensor_scalar(identM[:], io[:], 0, None, op0=OP.is_equal)
        nc.vector.tensor_scalar(identS[:], io[:], 0, None, op0=OP.is_equal)
        nc.vector.tensor_scalar_mul(identM[:], identM[:], -1.0)
        nc.vector.tensor_scalar_mul(identS[:], identS[:], 0.5)
        negone = small.tile([128, 1], FP32)
        nc.vector.memset(negone[:], -1.0)

        # ---- matmul1: h[e, tb] = W1 @ x.T, fused 0.5*h + 0.5*b1 into scan
        # layout. h_sb free index = t*512 + j*64 + b ----
        h_sb = persist.tile([128, T * 512], BF16)
        h3 = h_sb[:].rearrange("p (t x) -> p t x", x=512)

        def mm1_group(g):
            t0 = 8 * g
            for j in range(8):
                ps = ps_h.tile([128, 512], FP32, tag="ps_h", name=f"psh_{g}_{j}")
                for dp in range(4):   # pairs of contraction tiles (DoubleRow)
                    nc.tensor.matmul(
                        ps[:],
                        w1t3[:, 2 * dp:2 * dp + 2, j * 128:(j + 1) * 128],
                        xt4[:, g, 2 * dp:2 * dp + 2, :],
                        start=(dp == 0), stop=(dp == 3),
                        perf_mode=mybir.MatmulPerfMode.DoubleRow,
                    )
                nc.scalar.activation(
                    h3[:, t0:t0 + 8, j * 64:(j + 1) * 64],
                    ps[:].rearrange("p (t b) -> p t b", t=8),
                    AF.Identity, scale=0.5 / W1_PRESCALE, bias=b1h[:, j:j + 1],
                )

        # ---- LIF scan pieces (emitted interleaved with mm1 groups) ----
        lif = _lif_op()
        wst = [small.tile([128, 512], BF16, name=f"wst{i}") for i in range(2)]
        nc.vector.memset(wst[0][:], 0.0)
        m_tiles = {}

        def scan_steps(t0, t1):
            for t in range(t0, t1):
                prev, cur = wst[t % 2], wst[(t + 1) % 2]
                nc.vector._custom_dve(lif, out=cur[:], in0=prev[:],
                                      in1=h3[:, t, :], s0=1.0, s1=0.5)
                m = mpool.tile([128, 512], BF16, tag="m", name=f"m{t}")
                m_tiles[t] = m
                if t < 24:
                    nc.vector.tensor_scalar(m[:], cur[:], 1.0, None, op0=OP.is_lt)
                else:
                    # ScalarE is idle by then; sign(w-1) = 2s-1 rides the same
                    # accumulation with the +0.5-scaled identity
                    nc.scalar.activation(m[:], cur[:], AF.Sign, bias=negone[:, 0:1])

        def msum_steps(t0, t1):
            # one 213ns identity matmul per step; also keeps the PE duty
            # cycle up through the scan tail (HAM stays at full clock)
            for t in range(t0, t1):
                tgt = msumA if t < 16 else msumB
                idv = identM if t < 24 else identS
                nc.tensor.matmul(tgt[:], idv[:], m_tiles[t][:],
                                 start=(t in (0, 16)), stop=(t in (15, 31)))

        ones = small.tile([1, BC], BF16)
        nc.vector.memset(ones[:], 1.0)

        # ---- emission: mm1 groups + interleaved scan / spike sums; mm2a
        # goes after g3 so it fills the PE while the scan tail runs ----
        mm1_group(0)
        mm1_group(1)
        scan_steps(0, 8)
        msum_steps(0, 6)
        mm1_group(2)
        scan_steps(8, 16)
        msum_steps(6, 16)          # completes msumA (t 0..15)
        mm1_group(3)
        scan_steps(16, 24)

        b2_32 = small.tile([1, C], BF16)
        nc.scalar.activation(b2_32[:], b2_sb[:], AF.Copy,
                             scale=float(T) * W1_PRESCALE)
        ssumA = small.tile([128, 512], FP8)
        nc.scalar.activation(ssumA[:], msumA[:], AF.Copy, scale=1.0, bias=16.0)
        ssumA3 = ssumA[:].rearrange("p (j b) -> p j b", j=8)

        y_sb = small.tile([BC, 1024], FP32)
        psy = [ps_y.tile([BC, 512], FP32, tag="ps_y", name=f"psy{h}")
               for h in range(2)]

        def mm2(ssum3, first, last):
            for half in range(2):
                n = 512 if half == 0 else C - 512
                c0 = half * 512
                for pj in range(4):   # DoubleRow pairs of e-tiles
                    nc.tensor.matmul(
                        psy[half][:, 0:n],
                        ssum3[:, 2 * pj:2 * pj + 2, :],
                        w2t3[:, 2 * pj:2 * pj + 2, c0:c0 + n],
                        start=(first and pj == 0),
                        stop=(last and pj == 3),
                        perf_mode=mybir.MatmulPerfMode.DoubleRow,
                    )
                if first:   # bias rides in the first accumulation pass only
                    nc.tensor.matmul(psy[half][:, 0:n], ones[:],
                                     b2_32[:, c0:c0 + n], start=False, stop=False)

        mm2(ssumA3, True, False)   # runs while the scan continues
        msum_steps(16, 24)
        scan_steps(24, 32)
        msum_steps(24, 32)         # completes msumB (t 16..31)

        ssumB = small.tile([128, 512], FP8)
        nc.scalar.activation(ssumB[:], msumB[:], AF.Copy, scale=1.0, bias=12.0)
        ssumB3 = ssumB[:].rearrange("p (j b) -> p j b", j=8)
        mm2(ssumB3, False, True)

        # ---- log_softmax over C. |y| <= ~35 so no max-shift needed; the
        # Exp's free-dim accumulator replaces a separate reduce. ----
        ez = small.tile([BC, 1024], BF16)
        esum = small.tile([BC, 2], FP32)
        esum1 = small.tile([BC, 1], FP32)
        lse = small.tile([BC, 1], FP32)
        out_sb = small.tile([BC, C], FP32)
        for half in range(2):
            n = 512 if half == 0 else C - 512
            c0 = half * 512
            nc.scalar.activation(y_sb[:, c0:c0 + n], psy[half][:, 0:n], AF.Copy,
                                 scale=1.0 / (T * W1_PRESCALE))
            nc.scalar.activation(ez[:, c0:c0 + n], y_sb[:, c0:c0 + n], AF.Exp,
                                 accum_out=esum[:, half:half + 1])
        nc.vector.tensor_tensor(esum1[:], esum[:, 0:1], esum[:, 1:2], op=OP.add)
        nc.scalar.activation(lse[:], esum1[:], AF.Ln)
        for half, ring in ((0, nc.sync), (1, nc.scalar)):
            n = 512 if half == 0 else C - 512
            c0 = half * 512
            nc.vector.tensor_scalar(out_sb[:, c0:c0 + n], y_sb[:, c0:c0 + n],
                                    lse[:], None, op0=OP.subtract)
            ring.dma_start(y_d[:, c0:c0 + n], out_sb[:, c0:c0 + n])

    nc.compile()
    return nc


_CACHE = {}


def kernel(x, W1, b1, W2, b2):
    if "nc" not in _CACHE:
        _CACHE["nc"] = build_program()
    nc = _CACHE["nc"]

    f8 = mybir.dt.np(FP8)
    x = np.asarray(x, dtype=np.float32)
    w1t = np.ascontiguousarray(
        (np.asarray(W1, dtype=np.float32).T * W1_PRESCALE).astype(f8)
    ).reshape(128, 8 * 1024)          # d = p*8+dj on partitions
    w2t = np.ascontiguousarray(
        (np.asarray(W2, dtype=np.float32).T * W1_PRESCALE).astype(f8))
    b1 = np.ascontiguousarray(b1, dtype=np.float32)
    b2 = np.ascontiguousarray(b2, dtype=np.float32)
    in_maps = []
    for i in range(N_CORES):
        xs = x[:, i * BC:(i + 1) * BC, :].reshape(TB, D).T.astype(f8)
        # [d, tb] -> [p, k, dj, t'] with d = p*8+dj, tb = k*512+t'
        # (chunk-major per partition: matches the SBUF tile layout exactly,
        #  so the two load DMAs are fully contiguous on both sides)
        xs4 = xs.reshape(128, 8, NCH, 512).transpose(0, 2, 1, 3)
        xhost = np.ascontiguousarray(xs4).reshape(128, NCH * 4096)
        in_maps.append({"xT": xhost, "W1T": w1t, "b1": b1, "W2T": w2t, "b2": b2})

    res = run_bass_kernel_spmd(nc, in_maps, core_ids=list(range(N_CORES)),
                               **_CACHE.get("run_kwargs", {}))
    _CACHE["last_results"] = res
    out = np.concatenate([res.results[i]["y"] for i in range(N_CORES)], axis=0)
    return out
